# revision 1
# baseline (speedup 1.0000x reference)
"""DCN (cross+deep) Trainium2 Bass kernel, 8 NeuronCores.

Sharding: data-parallel over batch (2048 rows/core); embedding table
replicated in each core's HBM (bf16) and gathered on-device via indirect
DMA; cross/deep weights replicated.

Per-core dataflow (batch processed in 4 chunks of 512):
  gather [128,896]x4 (26 features + 2 pad-feature gathers of a zero row)
  -> feature_value scale (DVE) -> store natural chunk to DRAM scratch
  -> 7x DMA-transpose loads -> xT [896(7 ptiles), 512] bf16
  deep:  3 dense layers, PE matmuls (bf16, f32 PSUM), ACT relu+bias
  cross: S_i = w_i . y (PE matvec with column-replicated lhsT so PSUM holds
         S broadcast across partitions), DVE elementwise updates.
         cross_b constants are folded algebraically: y_i = yhat_i + C_i
         with C_i = sum_{j<i} cb_j, so only yhat is materialized; the
         correction enters via sigma_i = C_i * sum(w_i) (ACT bias) and a
         final output constant.
  out:   9 accumulating matvecs over [y_cross ; y_deep], + (out_b + C_3*sum(ow_c)).
"""

import numpy as np
import ml_dtypes
from contextlib import ExitStack

import concourse.tile as tile
import concourse.mybir as mybir
from concourse import bacc
from concourse.bass_utils import run_bass_kernel_spmd

# ---- problem constants (hardcoded; kernel.py must be self-contained) ----
B, F, E = 16384, 26, 32
NF = 1_000_000
D = F * E                    # 832
DEEP = (1024, 512, 256)
N_CROSS = 3
N_CORES = 8
S = B // N_CORES             # 2048 batch rows per core
FP = F + 2                   # features padded with 2 zero-row gathers
DP = FP * E                  # 896 = 7*128
KT = DP // 128               # 7
CHUNK = 512
NCHUNK = S // CHUNK          # 4
SUB = 128
SUBC = CHUNK // SUB          # 4
NSUB = S // SUB              # 16
M0, M1, M2 = DEEP[0] // 128, DEEP[1] // 128, DEEP[2] // 128  # 8, 4, 2

_bf = mybir.dt.bfloat16
_f32 = mybir.dt.float32
_i32 = mybir.dt.int32
_np_bf = ml_dtypes.bfloat16

_CACHE = {}
DEBUG = False
# pool-depth tuning knobs (swept against the cost-model timeline sim)
CFG = dict(xp=2, yp=2, cp=3, spp=3, dps=3, sps=2, po=2)


def _build_nc(with_fv=True):
    AF = mybir.ActivationFunctionType
    OP = mybir.AluOpType
    nc = bacc.Bacc(
        "TRN2", target_bir_lowering=False, debug=False, num_devices=N_CORES
    )

    # gathered embedding rows (host gather), natural layout [batch, 896]
    xn_d = nc.dram_tensor("xnat", [S, DP], _bf, kind="ExternalInput")
    # feature_value pre-transposed on host into the xT domain:
    # fvT[p, k*S + b] = feature_value[b, (k*128+p)//E]  (pad features -> 1.0)
    # When feature_value is identically 1.0 (the common case), the `with_fv=False`
    # specialization drops this input and the per-tile multiplies.
    if with_fv:
        fv_d = nc.dram_tensor("fv", [128, KT * S], _bf, kind="ExternalInput")
    w0_d = nc.dram_tensor("w0", [DP, DEEP[0]], _bf, kind="ExternalInput")
    w1_d = nc.dram_tensor("w1", [DEEP[0], DEEP[1]], _bf, kind="ExternalInput")
    w2_d = nc.dram_tensor("w2", [DEEP[1], DEEP[2]], _bf, kind="ExternalInput")
    cwb_d = nc.dram_tensor("cwb", [128, N_CROSS * KT * 128], _bf, kind="ExternalInput")
    # merged f32 constants: [b0(8) | b1(4) | b2(2) | sig(2) | ob(1)] = 17 cols
    cst_d = nc.dram_tensor("cst", [128, M0 + M1 + M2 + 3], _f32, kind="ExternalInput")
    ow_d = nc.dram_tensor("ow", [128, KT + M2], _bf, kind="ExternalInput")
    out_d = nc.dram_tensor("out", [S, 1], _f32, kind="ExternalOutput")
    if DEBUG:
        dbg_xt = nc.dram_tensor("dbg_xt", [128, CHUNK], _bf, kind="ExternalOutput")
        dbg_y0 = nc.dram_tensor("dbg_y0", [128, CHUNK], _bf, kind="ExternalOutput")
        dbg_s0 = nc.dram_tensor("dbg_s0", [128, CHUNK], _bf, kind="ExternalOutput")
        dbg_yc = nc.dram_tensor("dbg_yc", [128, CHUNK], _bf, kind="ExternalOutput")

    with ExitStack() as ctx:
        tc = ctx.enter_context(tile.TileContext(nc))
        wp = ctx.enter_context(tc.tile_pool(name="wp", bufs=1))
        xp = ctx.enter_context(tc.tile_pool(name="xp", bufs=CFG["xp"]))
        yp = ctx.enter_context(tc.tile_pool(name="yp", bufs=CFG["yp"]))
        cp = ctx.enter_context(tc.tile_pool(name="cp", bufs=CFG["cp"]))
        spp = ctx.enter_context(tc.tile_pool(name="spp", bufs=CFG["spp"]))
        otp = ctx.enter_context(tc.tile_pool(name="otp", bufs=2))
        dps = ctx.enter_context(tc.tile_pool(name="dps", bufs=CFG["dps"], space="PSUM"))
        sps = ctx.enter_context(tc.tile_pool(name="sps", bufs=CFG["sps"], space="PSUM"))
        ops = ctx.enter_context(tc.tile_pool(name="ops", bufs=CFG["po"], space="PSUM"))

        # ---- weights / constants to SBUF (once) ----
        # Emission order ~ schedule priority: first the tensors chunk 0 needs
        # (consts, w0, chunk-0 x slices + fv slices), then the late-use
        # weights (w1/w2/cwb/ow) so their DMA time hides under L1 compute.
        cst_sb = wp.tile([128, M0 + M1 + M2 + 3], _f32)
        nc.sync.dma_start(cst_sb[:], cst_d[:, :])
        b0_sb = cst_sb[:, 0:M0]
        b1_sb = cst_sb[:, M0:M0 + M1]
        b2_sb = cst_sb[:, M0 + M1:M0 + M1 + M2]
        sig_sb = cst_sb[:, M0 + M1 + M2:M0 + M1 + M2 + 2]
        ob_sb = cst_sb[:, M0 + M1 + M2 + 2:M0 + M1 + M2 + 3]
        w0_sb = wp.tile([128, KT, DEEP[0]], _bf)
        w0_r = w0_d[:, :].rearrange("(k p) m -> p k m", p=128)
        nc.sync.dma_start(w0_sb[:, :, 0:512], w0_r[:, :, 0:512])
        if with_fv:
            fv_sb = wp.tile([128, KT * S], _bf)
            nc.sync.dma_start(fv_sb[:], fv_d[:, :])
        w1_sb = wp.tile([128, M0, DEEP[1]], _bf)
        w2_sb = wp.tile([128, M1, DEEP[2]], _bf)
        cwb_sb = wp.tile([128, N_CROSS * KT * 128], _bf)
        ow_sb = wp.tile([128, KT + M2], _bf)

        def _late_loads():
            nc.sync.dma_start(w0_sb[:, :, 512:1024], w0_r[:, :, 512:1024])
            nc.sync.dma_start(w1_sb[:], w1_d[:, :].rearrange("(k p) m -> p k m", p=128))
            nc.sync.dma_start(w2_sb[:], w2_d[:, :].rearrange("(k p) m -> p k m", p=128))
            nc.sync.dma_start(cwb_sb[:], cwb_d[:, :])
            nc.sync.dma_start(ow_sb[:], ow_d[:, :])

        # "Observe" ops: each engine touches its DMA-loaded constants once so
        # steady-state instructions carry at most one semaphore wait (several
        # instruction encodings only have room for a single sync wait).
        obs = wp.tile([128, 8], _f32)
        obs_b = wp.tile([128, 8], _bf)
        if with_fv:
            nc.vector.tensor_copy(obs_b[:, 0:1], fv_sb[:, 0:1])
        nc.vector.tensor_copy(obs[:, 0:1], ob_sb[:, 0:1])
        nc.scalar.activation(obs[:, 1:2], b0_sb[:, 0:1], AF.Copy)
        nc.scalar.activation(obs[:, 2:3], b1_sb[:, 0:1], AF.Copy)
        nc.scalar.activation(obs[:, 3:4], b2_sb[:, 0:1], AF.Copy)
        nc.scalar.activation(obs[:, 4:5], sig_sb[:, 0:1], AF.Copy)
        # PE warm-up burst: keep the PE busy during the startup DMA window so
        # the HAM clock-gate reaches 8/8 before the first real matmul group.
        warm = wp.tile([128, 512], _bf)
        nc.gpsimd.memset(warm[:], 0.0)
        warm_ps = dps.tile([128, 512], _f32, tag="dps", name="warm_ps")
        for _ in range(8):
            nc.tensor.matmul(
                warm_ps[:], lhsT=warm[:, 0:128], rhs=warm[:], start=True, stop=True
            )
        dummy_ps = ops.tile([1, 8], _f32, tag="dummy", bufs=1)
        for w_ap in (
            w0_sb[:, 0, 0:1],
            w1_sb[:, 0, 0:1],
            w2_sb[:, 0, 0:1],
            cwb_sb[:, 0:1],
            ow_sb[:, 0:1],
        ):
            nc.tensor.matmul(dummy_ps[0:1, 0:1], lhsT=w_ap, rhs=w_ap, start=True, stop=True)

        for c in range(NCHUNK):
            # ---- transposed loads + feature_value scale (in the xT domain) ----
            xT = []
            for k in range(KT):
                t = xp.tile([128, CHUNK], _bf, tag=f"xT{k}", name=f"xT{k}_{c}")
                nc.sync.dma_start(
                    out=t[:],
                    in_=xn_d[c * CHUNK:(c + 1) * CHUNK, k * 128:(k + 1) * 128],
                    transpose=True,
                )
                if with_fv:
                    nc.vector.tensor_tensor(
                        out=t[:],
                        in0=t[:],
                        in1=fv_sb[:, k * S + c * CHUNK:k * S + (c + 1) * CHUNK],
                        op=OP.mult,
                    )
                xT.append(t)
            if c == 0:
                _late_loads()
            if DEBUG and c == 0:
                nc.sync.dma_start(out=dbg_xt[:, :], in_=xT[0][:])

            # ---- cross branch (yhat formulation) ----
            yc = xT
            for i in range(N_CROSS):
                pss = sps.tile([128, CHUNK], _f32, tag="sps", name=f"s_{c}_{i}")
                for k in range(KT):
                    col = (i * KT + k) * 128
                    nc.tensor.matmul(
                        pss[:],
                        lhsT=cwb_sb[:, col:col + 128],
                        rhs=yc[k][:],
                        start=(k == 0),
                        stop=(k == KT - 1),
                    )
                sp_t = spp.tile([128, CHUNK], _bf, tag="sp", name=f"sp_{c}_{i}")
                if i == 0:
                    # S0' = S0 + 1   (yhat1 = x0 * (S0 + 1))
                    nc.scalar.activation(sp_t[:], pss[:], AF.Copy, bias=1.0)
                else:
                    # Si' = Si + sigma_i
                    nc.scalar.activation(
                        sp_t[:], pss[:], AF.Identity, bias=sig_sb[:, i - 1:i]
                    )
                newyc = []
                for k in range(KT):
                    nt = cp.tile([128, CHUNK], _bf, tag=f"yc{k}", name=f"yc{i}_{c}_{k}")
                    if i == 0:
                        nc.vector.tensor_tensor(
                            out=nt[:], in0=xT[k][:], in1=sp_t[:], op=OP.mult
                        )
                    else:
                        tt = cp.tile(
                            [128, CHUNK], _bf, tag="tmp", name=f"tmp_{c}_{i}_{k}"
                        )
                        nc.vector.tensor_tensor(
                            out=tt[:], in0=xT[k][:], in1=sp_t[:], op=OP.mult
                        )
                        nc.vector.tensor_tensor(
                            out=nt[:], in0=tt[:], in1=yc[k][:], op=OP.add
                        )
                    newyc.append(nt)
                if DEBUG and c == 0 and i == 0:
                    nc.sync.dma_start(out=dbg_s0[:, :], in_=sp_t[:])
                yc = newyc
            if DEBUG and c == 0:
                nc.sync.dma_start(out=dbg_yc[:, :], in_=yc[0][:])

            # ---- deep branch ----
            y0 = []
            for m in range(M0):
                ps = dps.tile([128, CHUNK], _f32, tag="dps", name=f"ps0_{c}_{m}")
                for k in range(KT):
                    nc.tensor.matmul(
                        ps[:],
                        lhsT=w0_sb[:, k, m * 128:(m + 1) * 128],
                        rhs=xT[k][:],
                        start=(k == 0),
                        stop=(k == KT - 1),
                    )
                t = yp.tile([128, CHUNK], _bf, tag=f"y0_{m}", name=f"y0_{c}_{m}")
                nc.scalar.activation(t[:], ps[:], AF.Relu, bias=b0_sb[:, m:m + 1])
                y0.append(t)
            if DEBUG and c == 0:
                nc.sync.dma_start(out=dbg_y0[:, :], in_=y0[0][:])
            y1 = []
            for m in range(M1):
                ps = dps.tile([128, CHUNK], _f32, tag="dps", name=f"ps1_{c}_{m}")
                for k in range(M0):
                    nc.tensor.matmul(
                        ps[:],
                        lhsT=w1_sb[:, k, m * 128:(m + 1) * 128],
                        rhs=y0[k][:],
                        start=(k == 0),
                        stop=(k == M0 - 1),
                    )
                t = yp.tile([128, CHUNK], _bf, tag=f"y1_{m}", name=f"y1_{c}_{m}")
                nc.scalar.activation(t[:], ps[:], AF.Relu, bias=b1_sb[:, m:m + 1])
                y1.append(t)
            y2 = []
            for m in range(M2):
                ps = dps.tile([128, CHUNK], _f32, tag="dps", name=f"ps2_{c}_{m}")
                for k in range(M1):
                    nc.tensor.matmul(
                        ps[:],
                        lhsT=w2_sb[:, k, m * 128:(m + 1) * 128],
                        rhs=y1[k][:],
                        start=(k == 0),
                        stop=(k == M1 - 1),
                    )
                t = yp.tile([128, CHUNK], _bf, tag=f"y2_{m}", name=f"y2_{c}_{m}")
                nc.scalar.activation(t[:], ps[:], AF.Relu, bias=b2_sb[:, m:m + 1])
                y2.append(t)

            # ---- output layer: concat matvec ----
            po = ops.tile([1, CHUNK], _f32, tag="po", name=f"po_{c}")
            srcs = yc + y2
            for j, src in enumerate(srcs):
                nc.tensor.matmul(
                    po[:],
                    lhsT=ow_sb[:, j:j + 1],
                    rhs=src[:],
                    start=(j == 0),
                    stop=(j == len(srcs) - 1),
                )
            ot = otp.tile([1, CHUNK], _f32, tag="ot", name=f"ot_{c}")
            nc.vector.tensor_scalar_add(ot[:], po[:], ob_sb[0:1, 0:1])
            nc.sync.dma_start(
                out=out_d[c * CHUNK:(c + 1) * CHUNK, :].rearrange("n o -> o n"),
                in_=ot[:],
            )

    nc.compile()
    return nc


def _get_nc(with_fv=True):
    key = f"nc_fv{int(with_fv)}"
    if key not in _CACHE:
        _CACHE[key] = _build_nc(with_fv=with_fv)
    return _CACHE[key]


def _prep_in_maps(inputs, with_fv=True):
    fi = np.asarray(inputs["feature_index"]).astype(np.int64)
    fvv = np.asarray(inputs["feature_value"], dtype=np.float32)
    emb = np.asarray(inputs["emb_table"])
    cw = np.asarray(inputs["cross_w"], dtype=np.float32)
    cb = np.asarray(inputs["cross_b"], dtype=np.float32)
    w0 = np.asarray(inputs["w0"], dtype=np.float32)
    b0 = np.asarray(inputs["b0"], dtype=np.float32)
    w1 = np.asarray(inputs["w1"], dtype=np.float32)
    b1 = np.asarray(inputs["b1"], dtype=np.float32)
    w2 = np.asarray(inputs["w2"], dtype=np.float32)
    b2 = np.asarray(inputs["b2"], dtype=np.float32)
    ow = np.asarray(inputs["out_w"], dtype=np.float32).reshape(-1)
    ob = np.asarray(inputs["out_b"], dtype=np.float32).reshape(-1)

    # shared (replicated) tensors
    table = np.zeros((NF + 1, E), dtype=_np_bf)
    table[:NF] = emb.astype(_np_bf)
    # host-side gather (padded features hit the zero row NF)
    idxp = np.full((B, FP), NF, dtype=np.int64)
    idxp[:, :F] = fi
    xnat_all = table[idxp].reshape(B, DP)  # bf16 [B, 896]
    w0p = np.zeros((DP, DEEP[0]), dtype=_np_bf)
    w0p[:D] = w0.astype(_np_bf)
    w1b = np.ascontiguousarray(w1.astype(_np_bf))
    w2b = np.ascontiguousarray(w2.astype(_np_bf))
    cwp = np.zeros((N_CROSS, DP), dtype=np.float32)
    cwp[:, :D] = cw
    # cwb[p, (i*KT+k)*128 + j] = cw[i, k*128+p]  (replicated along free dim j)
    cwb = np.zeros((128, N_CROSS * KT * 128), dtype=_np_bf)
    for i in range(N_CROSS):
        for k in range(KT):
            seg = cwp[i, k * 128:(k + 1) * 128].astype(_np_bf)
            cwb[:, (i * KT + k) * 128:(i * KT + k + 1) * 128] = seg[:, None]
    b0r = b0.reshape(M0, 128).T.astype(np.float32)
    b1r = b1.reshape(M1, 128).T.astype(np.float32)
    b2r = b2.reshape(M2, 128).T.astype(np.float32)
    C = np.cumsum(cb)  # C[i] = cb_0 + ... + cb_i
    sig = np.zeros((128, 2), dtype=np.float32)
    sig[:, 0] = C[0] * cw[1].sum()
    sig[:, 1] = C[1] * cw[2].sum()
    owp = np.zeros((DP + DEEP[2],), dtype=np.float32)
    owp[:D] = ow[:D]
    owp[DP:] = ow[D:]
    ow_arr = np.ascontiguousarray(owp.reshape(KT + M2, 128).T.astype(_np_bf))
    obt = np.full((128, 1), ob[0] + C[2] * ow[:D].sum(), dtype=np.float32)
    cst = np.ascontiguousarray(
        np.concatenate([b0r, b1r, b2r, sig, obt], axis=1).astype(np.float32)
    )

    shared = dict(w0=w0p, w1=w1b, w2=w2b, cwb=cwb, cst=cst, ow=ow_arr)

    in_maps = []
    for core in range(N_CORES):
        xnat = np.ascontiguousarray(xnat_all[core * S:(core + 1) * S])
        m = dict(xnat=xnat, **shared)
        if with_fv:
            fvc = fvv[core * S:(core + 1) * S]  # [S, F]
            fvp = np.ones((S, FP), dtype=np.float32)
            fvp[:, :F] = fvc
            # fvT[p, k*S + b] = fvp[b, (k*128+p)//E]
            fve = np.repeat(fvp, E, axis=1)          # [S, DP]
            fvT = fve.T.reshape(KT, 128, S).transpose(1, 0, 2).reshape(128, KT * S)
            m["fv"] = np.ascontiguousarray(fvT.astype(_np_bf))
        in_maps.append(m)
    return in_maps


def _run(inputs, trace=False, **kw):
    fvv = np.asarray(inputs["feature_value"], dtype=np.float32)
    with_fv = not bool(np.all(fvv == 1.0))
    nc = _get_nc(with_fv=with_fv)
    in_maps = _prep_in_maps(inputs, with_fv=with_fv)
    res = run_bass_kernel_spmd(
        nc, in_maps, core_ids=list(range(N_CORES)), trace=trace, **kw
    )
    out = np.concatenate([r["out"] for r in res.results], axis=0)
    return out.astype(np.float32), res


def kernel(**inputs) -> np.ndarray:
    out, _ = _run(inputs, trace=False)
    return out



# revision 2
# speedup vs baseline: 2.5269x; 2.5269x over previous
"""DCN (cross+deep) Trainium2 Bass kernel, 8 NeuronCores.

Sharding: data-parallel over batch (2048 rows/core); embedding gather +
fp8 quantization + layout done host-side; cross/deep weights replicated.

Per-core dataflow (batch in 4 chunks of 512):
  deep:  fp8e4m3 DoubleRow matmuls (2 k-tiles per instruction, 0.5
         cyc/row) for all 3 layers; x scaled by s_x=128, weights by
         s_w=32, hidden activations re-quantized to fp8 (s_y=128) by the
         relu stage (DVE tensor_scalar mult+max for L0 with bias folded
         into a constant x column; ACT scale+bias+relu for L1/L2);
         y2 kept bf16.
  cross: collapsed algebraically. y_i = yhat_i + C_i with yhat_i =
         x0 * tau_i (per-row scalar), tau_{i+1} = tau_i*(S_i+1) + sig_i,
         S_i = cross_w_i . x0, sig_i = C_i*sum(w_i). Output-layer cross
         part = tau_3 * P with P = out_w_cross . x0. The 4 dots are
         computed as stationary-x matmuls (lhsT = x block [128d,128b],
         rhs = packed weights [128,4]) -- out free size 4, so nearly
         free on PE. Dot precision is recovered with a 3-product hi/lo
         fp8 decomposition: xh@cwh + xh@cwl + xl@cwh2 where
         xl = fp8((x*s_x - xh)*16), cwl = fp8(cw*s_cw - cwh),
         cwh2 = fp8(cw*s_cw/16).
  out:   po = ow_deep . y2 (bf16 matvec, out free 1); recursion + final
         combine on DVE over [128,4] tiles (partition = batch%128,
         free = batch-block); result [128,4] f32 scatter-DMA'd to
         out[2048,1].
"""

import numpy as np
import ml_dtypes
from contextlib import ExitStack

import concourse.tile as tile
import concourse.mybir as mybir
from concourse import bacc
from concourse.bass_utils import run_bass_kernel_spmd

# ---- problem constants (hardcoded; kernel.py must be self-contained) ----
B, F, E = 16384, 26, 32
NF = 1_000_000
D = F * E                    # 832
DEEP = (1024, 512, 256)
N_CROSS = 3
N_CORES = 8
S = B // N_CORES             # 2048 batch rows per core
DP = 1024                    # x padded to 8 k-planes (bias col at 832)
KP = DP // 128               # 8 x k-planes
KL = 7                       # x-lo planes (real dims only)
CHUNK = 512
NCHUNK = S // CHUNK          # 4
NBLK = CHUNK // 128          # 4 batch blocks per chunk
M0, M1, M2 = DEEP[0] // 128, DEEP[1] // 128, DEEP[2] // 128  # 8, 4, 2
NG = 3 * KL                  # dot matmul count per block (hi,wl,lo)

# scales (powers of two)
S_X = 128.0
S_W = 32.0
S_CW = 32.0
S_Y = 128.0
D0 = S_Y / (S_X * S_W)       # L0 psum -> y0*S_Y
D1 = S_Y / (S_Y * S_W)       # L1 psum -> y1*S_Y
D2 = 1.0 / (S_Y * S_W)       # L2 psum -> y2 (unscaled)
DD = 1.0 / (S_X * S_CW)      # dots descale

_bf = mybir.dt.bfloat16
_f8 = mybir.dt.float8e4
_f32 = mybir.dt.float32
_np_bf = ml_dtypes.bfloat16
_np_f8 = ml_dtypes.float8_e4m3

_CACHE = {}
# tuning knobs
CFG = dict(
    dve_l0=6,     # how many of L0's 8 relu tiles run on DVE (rest on ACT)
    warm=4,       # PE warm-up matmuls
    dps=3, ddp=2, pop=2,   # PSUM pool depths (banks)
    yp=2, cp=2,
    l1_wlo=False, l2_wlo=False,  # optional weight-lo products (accuracy)
    l0_wlo=False,
)


def _build_nc():
    AF = mybir.ActivationFunctionType
    OP = mybir.AluOpType
    DR = mybir.MatmulPerfMode.DoubleRow
    nc = bacc.Bacc(
        "TRN2", target_bir_lowering=False, debug=False, num_devices=N_CORES
    )

    # x-hi fp8, k-plane-major: xh[p, k*S + b] = x_hi[b, k*128+p]
    xh_d = nc.dram_tensor("xh", [128, KP * S], _f8, kind="ExternalInput")
    # x-lo fp8 (7 real planes): xl[p, k*S + b]
    xl_d = nc.dram_tensor("xl", [128, KL * S], _f8, kind="ExternalInput")
    # deep weights, m-major DR layout: w[p, (m*KK + plane)*128 + c]
    nw0 = 2 if CFG["l0_wlo"] else 1
    nw1 = 2 if CFG["l1_wlo"] else 1
    nw2 = 2 if CFG["l2_wlo"] else 1
    w0_d = nc.dram_tensor("w0", [128, nw0 * M0 * KP * 128], _f8, kind="ExternalInput")
    w1_d = nc.dram_tensor("w1", [128, nw1 * M1 * M0 * 128], _f8, kind="ExternalInput")
    w2_d = nc.dram_tensor("w2", [128, nw2 * M2 * M1 * 128], _f8, kind="ExternalInput")
    # packed cross/out dot weights: cwd[p, g*4 + q], g: 7 hi | 7 wl | 7 lo
    cwd_d = nc.dram_tensor("cwd", [128, NG * 4], _f8, kind="ExternalInput")
    # deep out weights: owd[p, t] = ow[832 + t*128 + p]
    owd_d = nc.dram_tensor("owd", [128, M2], _bf, kind="ExternalInput")
    # f32 constants: [b1r(4) | b2r(2) | sig1 | sig2 | obp] = 9 cols
    cst_d = nc.dram_tensor("cst", [128, M1 + M2 + 3], _f32, kind="ExternalInput")
    out_d = nc.dram_tensor("out", [S, 1], _f32, kind="ExternalOutput")

    with ExitStack() as ctx:
        tc = ctx.enter_context(tile.TileContext(nc))
        wp = ctx.enter_context(tc.tile_pool(name="wp", bufs=1))
        yp = ctx.enter_context(tc.tile_pool(name="yp", bufs=CFG["yp"]))
        cp = ctx.enter_context(tc.tile_pool(name="cp", bufs=CFG["cp"]))
        otp = ctx.enter_context(tc.tile_pool(name="otp", bufs=2))
        dps = ctx.enter_context(tc.tile_pool(name="dps", bufs=CFG["dps"], space="PSUM"))
        ddp = ctx.enter_context(tc.tile_pool(name="ddp", bufs=CFG["ddp"], space="PSUM"))
        pop = ctx.enter_context(tc.tile_pool(name="pop", bufs=CFG["pop"], space="PSUM"))

        # ---- persistent SBUF tensors ----
        cst_sb = wp.tile([128, M1 + M2 + 3], _f32)
        nc.sync.dma_start(cst_sb[:], cst_d[:, :])
        b1_sb = cst_sb[:, 0:M1]
        b2_sb = cst_sb[:, M1:M1 + M2]
        sig1_sb = cst_sb[:, M1 + M2:M1 + M2 + 1]
        sig2_sb = cst_sb[:, M1 + M2 + 1:M1 + M2 + 2]
        obp_sb = cst_sb[:, M1 + M2 + 2:M1 + M2 + 3]
        cwd_sb = wp.tile([128, NG * 4], _f8)
        nc.sync.dma_start(cwd_sb[:], cwd_d[:, :])
        owd_sb = wp.tile([128, M2], _bf)
        nc.sync.dma_start(owd_sb[:], owd_d[:, :])

        xh_sb = wp.tile([128, KP, S], _f8)
        xl_sb = wp.tile([128, KL, S], _f8)
        w0_sb = wp.tile([128, nw0 * M0, KP, 128], _f8)
        w1_sb = wp.tile([128, nw1 * M1, M0, 128], _f8)
        w2_sb = wp.tile([128, nw2 * M2, M1, 128], _f8)

        xh_r = xh_d[:, :].rearrange("p (k b) -> p k b", k=KP)
        xl_r = xl_d[:, :].rearrange("p (k b) -> p k b", k=KL)
        w0_r = w0_d[:, :].rearrange("p (m k c) -> p m k c", m=nw0 * M0, k=KP)
        w1_r = w1_d[:, :].rearrange("p (m k c) -> p m k c", m=nw1 * M1, k=M0)
        w2_r = w2_d[:, :].rearrange("p (m k c) -> p m k c", m=nw2 * M2, k=M1)

        def _xh_load(c):
            nc.sync.dma_start(
                xh_sb[:, :, c * CHUNK:(c + 1) * CHUNK],
                xh_r[:, :, c * CHUNK:(c + 1) * CHUNK],
            )

        def _xl_load(c):
            nc.sync.dma_start(
                xl_sb[:, :, c * CHUNK:(c + 1) * CHUNK],
                xl_r[:, :, c * CHUNK:(c + 1) * CHUNK],
            )

        # DMA order: consts, xh-c0, w0 (m-split), w1, w2, xl-c0, then the
        # rest of x interleaved. Order == DMA_ENGINES service order.
        _xh_load(0)
        for m in range(nw0 * M0):
            nc.sync.dma_start(w0_sb[:, m, :, :], w0_r[:, m, :, :])
        for m in range(nw1 * M1):
            nc.sync.dma_start(w1_sb[:, m, :, :], w1_r[:, m, :, :])
        nc.sync.dma_start(w2_sb[:], w2_r[:, :, :, :])
        _xl_load(0)
        _xh_load(1)
        _xl_load(1)
        _xh_load(2)
        _xl_load(2)
        _xh_load(3)
        _xl_load(3)

        # "Observe" ops: each engine touches its DMA-loaded constants once so
        # steady-state instructions carry at most one semaphore wait.
        obs = wp.tile([128, 8], _f32)
        nc.vector.tensor_copy(obs[:, 0:1], sig1_sb)
        nc.scalar.activation(obs[:, 1:2], b1_sb[:, 0:1], AF.Copy)
        nc.scalar.activation(obs[:, 2:3], b2_sb[:, 0:1], AF.Copy)
        nc.scalar.activation(obs[:, 3:4], obp_sb, AF.Copy)
        nc.vector.tensor_copy(obs[:, 4:5], sig2_sb)

        # PE warm-up: keep PE busy during the startup DMA window.
        warm = wp.tile([128, 512], _bf)
        nc.gpsimd.memset(warm[:], 0.0)
        warm_ps = dps.tile([128, 512], _f32, tag="dps", name="warm_ps")
        for _ in range(CFG["warm"]):
            nc.tensor.matmul(
                warm_ps[:], lhsT=warm[:, 0:128], rhs=warm[:], start=True, stop=True
            )
        # weight observes (single-wait rule): tiny matmuls into warm_ps.
        for w_ap in (
            w0_sb[:, 0, 0, 0:1],
            w1_sb[:, 0, 0, 0:1],
            w2_sb[:, 0, 0, 0:1],
            cwd_sb[:, 0:1],
        ):
            nc.tensor.matmul(
                warm_ps[0:1, 0:1], lhsT=w_ap, rhs=w_ap, start=True, stop=True
            )
        nc.tensor.matmul(
            warm_ps[0:1, 0:1],
            lhsT=owd_sb[:, 0:1],
            rhs=owd_sb[:, 0:1],
            start=True,
            stop=True,
        )

        for c in range(NCHUNK):
            cs = slice(c * CHUNK, (c + 1) * CHUNK)

            # ---- deep L0: fp8 DR, bias via constant x column ----
            y0 = yp.tile([128, M0 // 2, 2, CHUNK], _f8, tag="y0", name=f"y0_{c}")
            for m in range(M0):
                ps = dps.tile([128, CHUNK], _f32, tag="dps", name=f"ps0_{c}_{m}")
                np_ = KP // 2
                tot = nw0 * np_
                for g in range(tot):
                    wm = m if g < np_ else M0 + m
                    j = g % np_
                    nc.tensor.matmul(
                        ps[:],
                        lhsT=w0_sb[:, wm, 2 * j:2 * j + 2, :],
                        rhs=xh_sb[:, 2 * j:2 * j + 2, cs],
                        perf_mode=DR,
                        start=(g == 0),
                        stop=(g == tot - 1),
                        skip_group_check=True,
                    )
                dst = y0[:, m // 2, m % 2, :]
                if m < CFG["dve_l0"]:
                    nc.vector.tensor_scalar(
                        out=dst, in0=ps[:], scalar1=D0, scalar2=0.0,
                        op0=OP.mult, op1=OP.max,
                    )
                else:
                    nc.scalar.activation(dst, ps[:], AF.Relu, bias=0.0, scale=D0)

            # ---- deep L1 ----
            y1 = yp.tile([128, M1 // 2, 2, CHUNK], _f8, tag="y1", name=f"y1_{c}")
            for m in range(M1):
                ps = dps.tile([128, CHUNK], _f32, tag="dps", name=f"ps1_{c}_{m}")
                np_ = M0 // 2
                tot = nw1 * np_
                for g in range(tot):
                    wm = m if g < np_ else M1 + m
                    j = g % np_
                    nc.tensor.matmul(
                        ps[:],
                        lhsT=w1_sb[:, wm, 2 * j:2 * j + 2, :],
                        rhs=y0[:, j, :, :],
                        perf_mode=DR,
                        start=(g == 0),
                        stop=(g == tot - 1),
                        skip_group_check=True,
                    )
                nc.scalar.activation(
                    y1[:, m // 2, m % 2, :], ps[:], AF.Relu,
                    bias=b1_sb[:, m:m + 1], scale=D1,
                )

            # ---- deep L2 (y2 in bf16) ----
            y2 = yp.tile([128, M2, CHUNK], _bf, tag="y2", name=f"y2_{c}")
            for m in range(M2):
                ps = dps.tile([128, CHUNK], _f32, tag="dps", name=f"ps2_{c}_{m}")
                np_ = M1 // 2
                tot = nw2 * np_
                for g in range(tot):
                    wm = m if g < np_ else M2 + m
                    j = g % np_
                    nc.tensor.matmul(
                        ps[:],
                        lhsT=w2_sb[:, wm, 2 * j:2 * j + 2, :],
                        rhs=y1[:, j, :, :],
                        perf_mode=DR,
                        start=(g == 0),
                        stop=(g == tot - 1),
                        skip_group_check=True,
                    )
                nc.scalar.activation(
                    y2[:, m, :], ps[:], AF.Relu,
                    bias=b2_sb[:, m:m + 1], scale=D2,
                )

            # ---- deep out: po[b,1] per block (bf16 matvec) ----
            pps = pop.tile([128, 512], _f32, tag="pop", name=f"po_{c}")
            for jb in range(NBLK):
                bs = slice(c * CHUNK + jb * 128, c * CHUNK + (jb + 1) * 128)
                lbs = slice(jb * 128, (jb + 1) * 128)
                for t in range(M2):
                    nc.tensor.matmul(
                        pps[:, jb:jb + 1],
                        lhsT=y2[:, t, lbs],
                        rhs=owd_sb[:, t:t + 1],
                        start=(jb == 0 and t == 0),
                        stop=(jb == NBLK - 1 and t == M2 - 1),
                        skip_group_check=True,
                    )

            # ---- cross dots: S0,S1,S2,P per block (fp8, out free 4) ----
            dds = ddp.tile([128, 512], _f32, tag="ddp", name=f"dd_{c}")
            ddv = dds[:, 0:16].rearrange("p (j q) -> p j q", q=4)
            for jb in range(NBLK):
                bs = slice(c * CHUNK + jb * 128, c * CHUNK + (jb + 1) * 128)
                for g in range(NG):
                    k = g % KL
                    src = xh_sb if g < 2 * KL else xl_sb
                    nc.tensor.matmul(
                        ddv[:, jb, :],
                        lhsT=src[:, k, bs],
                        rhs=cwd_sb[:, g * 4:(g + 1) * 4],
                        start=(jb == 0 and g == 0),
                        stop=(jb == NBLK - 1 and g == NG - 1),
                        skip_group_check=True,
                    )

            # ---- tau recursion + combine (DVE, [128,4] tiles) ----
            dq = dds[:, 0:16].rearrange("p (j q) -> p q j", q=4)
            t1 = cp.tile([128, 4], _f32, tag="t1", name=f"t1_{c}")
            u1 = cp.tile([128, 4], _f32, tag="u1", name=f"u1_{c}")
            u2 = cp.tile([128, 4], _f32, tag="u2", name=f"u2_{c}")
            nc.vector.tensor_scalar(
                out=t1[:], in0=dq[:, 0, :], scalar1=DD, scalar2=1.0,
                op0=OP.mult, op1=OP.add,
            )
            nc.vector.tensor_scalar(
                out=u1[:], in0=dq[:, 1, :], scalar1=DD, scalar2=1.0,
                op0=OP.mult, op1=OP.add,
            )
            nc.vector.tensor_scalar(
                out=u2[:], in0=dq[:, 2, :], scalar1=DD, scalar2=1.0,
                op0=OP.mult, op1=OP.add,
            )
            ta = cp.tile([128, 4], _f32, tag="ta", name=f"ta_{c}")
            nc.vector.tensor_tensor(out=ta[:], in0=t1[:], in1=u1[:], op=OP.mult)
            tb = cp.tile([128, 4], _f32, tag="tb", name=f"tb_{c}")
            nc.vector.tensor_scalar(
                out=tb[:], in0=ta[:], scalar1=sig1_sb, scalar2=None, op0=OP.add
            )
            tc_ = cp.tile([128, 4], _f32, tag="tc", name=f"tc_{c}")
            nc.vector.tensor_tensor(out=tc_[:], in0=tb[:], in1=u2[:], op=OP.mult)
            t3 = cp.tile([128, 4], _f32, tag="t3", name=f"t3_{c}")
            nc.vector.tensor_scalar(
                out=t3[:], in0=tc_[:], scalar1=sig2_sb, scalar2=None, op0=OP.add
            )
            tq = cp.tile([128, 4], _f32, tag="tq", name=f"tq_{c}")
            nc.vector.tensor_tensor(out=tq[:], in0=t3[:], in1=dq[:, 3, :], op=OP.mult)
            tr = cp.tile([128, 4], _f32, tag="tr", name=f"tr_{c}")
            nc.vector.tensor_scalar(
                out=tr[:], in0=tq[:], scalar1=DD, scalar2=obp_sb,
                op0=OP.mult, op1=OP.add,
            )
            ot = otp.tile([128, 4], _f32, tag="ot", name=f"ot_{c}")
            nc.vector.tensor_tensor(out=ot[:], in0=tr[:], in1=pps[:, 0:4], op=OP.add)
            nc.sync.dma_start(
                out=out_d[cs, :].rearrange("(j p) o -> p (j o)", p=128),
                in_=ot[:],
            )

    nc.compile()
    return nc


def _get_nc():
    if "nc" not in _CACHE:
        _CACHE["nc"] = _build_nc()
    return _CACHE["nc"]


def _q8(a):
    return np.asarray(a, dtype=np.float32).astype(_np_f8)


def _prep_in_maps(inputs):
    fi = np.asarray(inputs["feature_index"]).astype(np.int64)
    fvv = np.asarray(inputs["feature_value"], dtype=np.float32)
    emb = np.asarray(inputs["emb_table"], dtype=np.float32)
    cw = np.asarray(inputs["cross_w"], dtype=np.float32)
    cb = np.asarray(inputs["cross_b"], dtype=np.float32)
    w0 = np.asarray(inputs["w0"], dtype=np.float32)
    b0 = np.asarray(inputs["b0"], dtype=np.float32)
    w1 = np.asarray(inputs["w1"], dtype=np.float32)
    b1 = np.asarray(inputs["b1"], dtype=np.float32)
    w2 = np.asarray(inputs["w2"], dtype=np.float32)
    b2 = np.asarray(inputs["b2"], dtype=np.float32)
    ow = np.asarray(inputs["out_w"], dtype=np.float32).reshape(-1)
    ob = np.asarray(inputs["out_b"], dtype=np.float32).reshape(-1)

    with_fv = not bool(np.all(fvv == 1.0))
    # ---- x gather + hi/lo fp8 quantization (host) ----
    if with_fv:
        xg = emb[fi] * fvv[:, :, None]                 # [B,F,E] f32
        xflat = xg.reshape(B, D) * S_X
        xh_all = np.zeros((B, DP), _np_f8)
        xh_all[:, :D] = xflat.astype(_np_f8)
        res = (xflat - xh_all[:, :D].astype(np.float32)) * 16.0
        xl_all = np.zeros((B, KL * 128), _np_f8)
        xl_all[:, :D] = res.astype(_np_f8)
    else:
        th = np.zeros((NF + 1, E), _np_f8)
        tscaled = emb * S_X
        th[:NF] = tscaled.astype(_np_f8)
        tl = np.zeros((NF + 1, E), _np_f8)
        tl[:NF] = ((tscaled - th[:NF].astype(np.float32)) * 16.0).astype(_np_f8)
        idxp = np.full((B, DP // E), NF, dtype=np.int64)
        idxp[:, :F] = fi
        xh_all = th[idxp].reshape(B, DP)
        xl_all = tl[idxp[:, :KL * 128 // E]].reshape(B, KL * 128)
    xh_all[:, D] = np.float32(S_X).astype(_np_f8)      # bias-one column (x=1*S_X)

    # ---- deep weights (m-major DR layout) ----
    nw0 = 2 if CFG["l0_wlo"] else 1
    nw1 = 2 if CFG["l1_wlo"] else 1
    nw2 = 2 if CFG["l2_wlo"] else 1

    def _wlayout(wq8_list, K, M):
        # wq8_list: list of [K*128, M*128] fp8 arrays (hi, optional lo)
        # -> [128, (len*M)*K*128] with m-major order (hi m's then lo m's)
        mats = np.concatenate([a.reshape(K, 128, M, 128) for a in wq8_list], axis=2)
        return np.ascontiguousarray(
            mats.transpose(1, 2, 0, 3).reshape(128, -1)
        )

    w0p = np.zeros((DP, DEEP[0]), np.float32)
    w0p[:D] = w0 * S_W
    w0p[D] = b0 * S_W                                   # bias row
    w0h = w0p.astype(_np_f8)
    w0l = [(w0p - w0h.astype(np.float32)).astype(_np_f8)] if CFG["l0_wlo"] else []
    w0_dr = _wlayout([w0h] + w0l, KP, M0)

    w1s = w1 * S_W
    w1h = w1s.astype(_np_f8)
    w1l = [(w1s - w1h.astype(np.float32)).astype(_np_f8)] if CFG["l1_wlo"] else []
    w1_dr = _wlayout([w1h] + w1l, M0, M1)

    w2s = w2 * S_W
    w2h = w2s.astype(_np_f8)
    w2l = [(w2s - w2h.astype(np.float32)).astype(_np_f8)] if CFG["l2_wlo"] else []
    w2_dr = _wlayout([w2h] + w2l, M1, M2)

    # ---- packed cross/out dot weights ----
    cwp = np.zeros((4, DP), np.float32)
    cwp[:N_CROSS, :D] = cw
    cwp[3, :D] = ow[:D]
    cwh = (cwp * S_CW).astype(_np_f8)
    cwl = (cwp * S_CW - cwh.astype(np.float32)).astype(_np_f8)
    cwh2 = (cwp * S_CW / 16.0).astype(_np_f8)
    cwd = np.zeros((128, NG * 4), _np_f8)
    for g in range(KL):
        cwd[:, g * 4:(g + 1) * 4] = cwh[:, g * 128:(g + 1) * 128].T
        cwd[:, (KL + g) * 4:(KL + g + 1) * 4] = cwl[:, g * 128:(g + 1) * 128].T
        cwd[:, (2 * KL + g) * 4:(2 * KL + g + 1) * 4] = cwh2[:, g * 128:(g + 1) * 128].T

    owd = np.ascontiguousarray(ow[D:].reshape(M2, 128).T.astype(_np_bf))

    # ---- f32 constants ----
    C = np.cumsum(cb)
    b1r = (S_Y * b1).reshape(M1, 128).T.astype(np.float32)
    b2r = b2.reshape(M2, 128).T.astype(np.float32)
    sig1 = np.full((128, 1), C[0] * cw[1].sum(), np.float32)
    sig2 = np.full((128, 1), C[1] * cw[2].sum(), np.float32)
    obp = np.full((128, 1), ob[0] + C[2] * ow[:D].sum(), np.float32)
    cst = np.ascontiguousarray(np.concatenate([b1r, b2r, sig1, sig2, obp], axis=1))

    shared = dict(w0=w0_dr, w1=w1_dr, w2=w2_dr, cwd=cwd, owd=owd, cst=cst)

    in_maps = []
    for core in range(N_CORES):
        rows = slice(core * S, (core + 1) * S)
        # [S, K*128] -> [128, K*S]: xdr[p, k*S+b] = x[b, k*128+p]
        xh8 = (
            xh_all[rows].view(np.uint8).reshape(S, KP, 128)
            .transpose(2, 1, 0).reshape(128, KP * S)
        )
        xl8 = (
            xl_all[rows].view(np.uint8).reshape(S, KL, 128)
            .transpose(2, 1, 0).reshape(128, KL * S)
        )
        m = dict(
            xh=np.ascontiguousarray(xh8).view(_np_f8),
            xl=np.ascontiguousarray(xl8).view(_np_f8),
            **shared,
        )
        in_maps.append(m)
    return in_maps


def _run(inputs, trace=False, **kw):
    nc = _get_nc()
    in_maps = _prep_in_maps(inputs)
    res = run_bass_kernel_spmd(
        nc, in_maps, core_ids=list(range(N_CORES)), trace=trace, **kw
    )
    out = np.concatenate([r["out"] for r in res.results], axis=0)
    return out.astype(np.float32), res


def kernel(**inputs) -> np.ndarray:
    out, _ = _run(inputs, trace=False)
    return out
